# revision 1
# baseline (speedup 1.0000x reference)
"""Trainium2 kernel for nn_LocalSorterModel (gnn_message_passing).

The reference model is entirely linear (pair-gather -> linear -> reshape ->
linear, no nonlinearity), so the whole network collapses exactly into a
single affine map:

    out[b, r] = sum_{n,d} embeds[b, n, d] * M[r, n*D + d] + const[r]

where M [120, 5120] and const [120] are cheap host-side precomputations from
the (small) weights:

    M[r, n*D+d] = sum_k W3[r,n,k] * w1[k,d] + sum_k W4[r,n,k] * w2[k,d]
    W3[r,n,k]   = sum_{p: IDX_I[p]=n} cls_w[r, p*D+k]   (W4 with IDX_J)
    const[r]    = sum_{p,k} pw_b[k] * cls_w[r, p*D+k] + cls_b[r]

Device work is then a single [2048, 5120] @ [5120, 120] matmul, memory-bound
on reading embeds. Sharding: 4 contraction slices x 2 batch halves across the
8 cores; each core streams its transposed X slice and M slice as fp16
(fp32 matmul runs at 1/4 PE rate and doubles DMA bytes; fp16 keeps rel err
at ~2e-4), accumulates in fp32 PSUM, and emits a [120, 1024] fp16 partial
which the host reduces in fp32, transposes, and biases.
"""

import numpy as np

import concourse.bacc as bacc
import concourse.mybir as mybir
from concourse.tile import TileContext
from concourse.bass_utils import run_bass_kernel_spmd

B = 2048          # batch
NI = 5            # items
D = 1024          # embed dim
KT = NI * D       # 5120 total contraction
R = 120           # num results
KF = 4            # contraction shards
BF = 2            # batch shards
NCORES = KF * BF  # 8
KC = KT // KF     # 1280 contraction per core
NCH = KC // 128   # 10 chunks of 128
BL = B // BF      # 1024 batch per core
NB = BL // 512    # 2 matmul column blocks

_f16 = mybir.dt.float16
_f32 = mybir.dt.float32

_CACHE = {}


def _build_nc(reps=1, w_eng="sync", w_split=False, x_eng="sync", o_eng="sync"):
    """reps>1 repeats the full DMA+matmul pipeline (bench-only) so device
    time can be measured as a slope; reps=1 is the production kernel."""
    nc = bacc.Bacc("TRN2", target_bir_lowering=False, debug=False)
    x = nc.dram_tensor("x", [NCH, 128, BL], _f16, kind="ExternalInput")
    w = nc.dram_tensor("w", [128, NCH * R], _f16, kind="ExternalInput")
    o = nc.dram_tensor("o", [R, BL], _f16, kind="ExternalOutput")

    def eng(name):
        return {"sync": nc.sync, "scalar": nc.scalar, "gpsimd": nc.gpsimd}[name]

    with TileContext(nc) as tc:
        with (
            tc.tile_pool(name="xp", bufs=min(2, reps) * NCH) as xp,
            tc.tile_pool(name="wp", bufs=min(2, reps)) as wp,
            tc.tile_pool(name="pp", bufs=1, space="PSUM") as pp,
            tc.tile_pool(name="op", bufs=NB) as op,
        ):
            ps = [
                pp.tile([R, 512], _f32, tag=f"ps{nb}", name=f"ps{nb}")
                for nb in range(NB)
            ]
            half = NCH // 2 * R
            for rep in range(reps):
                wt = wp.tile([128, NCH * R], _f16, tag="w", name="wt")
                if w_split:
                    eng(w_eng).dma_start(wt[:, :half], w[:, :half])
                else:
                    eng(w_eng).dma_start(wt[:], w[:, :])
                xts = []
                for c in range(NCH):
                    xt = xp.tile([128, BL], _f16, tag="x", name="xt")
                    eng(x_eng).dma_start(xt[:], x[c, :, :])
                    xts.append(xt)
                    if c == 0 and w_split:
                        eng(w_eng).dma_start(wt[:, half:], w[:, half:])

                for c in range(NCH):
                    for nb in range(NB):
                        nc.tensor.matmul(
                            ps[nb][:, :],
                            wt[:, c * R : (c + 1) * R],
                            xts[c][:, nb * 512 : (nb + 1) * 512],
                            start=(c == 0),
                            stop=(c == NCH - 1),
                            skip_group_check=True,
                        )
            for nb in range(NB):
                ot = op.tile([R, 512], _f16, tag="o")
                nc.vector.tensor_copy(ot[:], ps[nb][:])
                eng(o_eng).dma_start(o[:, nb * 512 : (nb + 1) * 512], ot[:])
    nc.compile()
    return nc


def _collapse_weights(pw_w, pw_b, cls_w, cls_b):
    """Exact linearization of the model -> (M_T [5120, 120] f32, const [120] f32)."""
    mask = ~np.eye(NI, dtype=bool)
    idx_i, idx_j = np.nonzero(mask)  # 20 ordered off-diagonal pairs, row-major

    cw = cls_w.reshape(R, NI * (NI - 1), D).astype(np.float64)
    w3 = np.zeros((R, NI, D))
    w4 = np.zeros((R, NI, D))
    for p in range(NI * (NI - 1)):
        w3[:, idx_i[p], :] += cw[:, p, :]
        w4[:, idx_j[p], :] += cw[:, p, :]
    w1 = pw_w[:, :D].astype(np.float64)
    w2 = pw_w[:, D:].astype(np.float64)
    m = w3.reshape(R * NI, D) @ w1 + w4.reshape(R * NI, D) @ w2  # [600, 1024]
    m = m.reshape(R, KT)
    const = cw.sum(axis=1) @ pw_b.astype(np.float64) + cls_b
    m_t = np.ascontiguousarray(m.T).astype(np.float32)  # [5120, 120]
    return m_t, const.astype(np.float32)


def kernel(embeds, pw_w, pw_b, cls_w, cls_b):
    embeds = np.asarray(embeds, dtype=np.float32)
    pw_w = np.asarray(pw_w, dtype=np.float32)
    pw_b = np.asarray(pw_b, dtype=np.float32)
    cls_w = np.asarray(cls_w, dtype=np.float32)
    cls_b = np.asarray(cls_b, dtype=np.float32)

    m_t, const = _collapse_weights(pw_w, pw_b, cls_w, cls_b)

    xf = embeds.reshape(B, KT)
    in_maps = []
    for core in range(NCORES):
        kf, bf = divmod(core, BF)
        x_c = np.ascontiguousarray(
            xf[bf * BL : (bf + 1) * BL, kf * KC : (kf + 1) * KC].T
        ).astype(np.float16).reshape(NCH, 128, BL)
        # w packed [128, NCH*R]: w[p, c*R + r] = M_T[kf*KC + c*128 + p, r]
        w_c = np.ascontiguousarray(
            m_t[kf * KC : (kf + 1) * KC, :]
            .reshape(NCH, 128, R)
            .transpose(1, 0, 2)
            .reshape(128, NCH * R)
        ).astype(np.float16)
        in_maps.append({"x": x_c, "w": w_c})

    if "nc" not in _CACHE:
        _CACHE["nc"] = _build_nc()
    res = run_bass_kernel_spmd(_CACHE["nc"], in_maps, core_ids=list(range(NCORES)))

    out = np.empty((B, R), dtype=np.float32)
    for bf in range(BF):
        acc = np.zeros((R, BL), dtype=np.float32)
        for kf in range(KF):
            acc += res.results[kf * BF + bf]["o"].astype(np.float32)
        out[bf * BL : (bf + 1) * BL, :] = acc.T
    out += const[None, :]
    return out

